# revision 38
# baseline (speedup 1.0000x reference)
"""Tensor-parallel Llama attention (GQA) on 8 TRN2 NeuronCores.

Strategy (v2):
  - Head-sharded QKV + attention: core m computes Q heads [4m, 4m+4) and
    KV head m (GQA group is exactly per-core, so no KV duplication).
  - All matmuls in bf16 with f32 PSUM accumulation (fp8 busts the 2e-2
    error budget: measured 2.6-4.5% per stage).
  - Transposed [feature, seq] layout keeps the PE contraction dim natural.
  - RoPE rotate_half via partition-shifted multiplies on the idle Pool
    engine with sign-folded sin constants (no PE matmul, no PSUM bank,
    no ACT copy: DVE/Pool read the projection PSUM directly).
  - Softmax without max-subtraction (scores O(17) << f32 exp overflow);
    row sums from an appended ones-column on V.
  - Causal trimming: diagonal key-tiles only compute/exp/mask/AV the
    valid query column range (saves ~15% of phase-2 work).
  - PE stream software-pipelined: score matmul kt+1 is emitted before
    the exp-dependent AV group of kt so the PE never waits on ACT;
    per-qc normalization is deferred past the next qc's first scores.
  - ACT does only exp; copies go to Pool/DVE (idle otherwise).
  - AllToAll per head converts head-sharding to sequence-sharding;
    o_proj runs in 4 hid-quarters (2 PSUM banks x 2 seq tiles each,
    double-buffered = 8 banks) with Wo streamed through a 20-deep pool.
  - Host gathers by concatenating the 8 [S/8, HID] outputs.
"""

import numpy as np
import ml_dtypes

H, KV, D, HID = 32, 8, 128, 4096
NCORES = 8
HPC = H // NCORES          # q heads per core
ROWS_Q = HPC * D           # q projection rows per core
P = 128
HD2 = D // 2
QCHUNK = 512               # attention q-chunk (score matmul free dim)
QS = 512                   # qkv-phase seq chunk
HQ = 1024                  # o_proj hid quarter width
ROPE_THETA = 10000.0
BF = ml_dtypes.bfloat16


def _patch_tile_drain():
    """This container's walrus build rejects a Drain instruction carrying
    semaphore waits ("Too many sync wait commands"). Re-emit the Tile tail
    drain's waits as standalone single-wait SP instructions, which the
    same walrus accepts, followed by a wait-free drain."""
    from concourse.tile import TileContext
    from concourse.vector_clock import ScopedClock

    if getattr(TileContext, "_drain_waits_patched", False):
        return

    def _drain_and_barrier(self, tick_clock, wait_clock):
        nc = self.nc
        probe = nc.sync.drain()
        wait_clock.add_sem_waits(
            probe.ins, ScopedClock({None: tick_clock.global_clock})
        )
        waits = list(probe.ins.sync_info.on_wait)
        probe.ins.sync_info.on_wait = []
        id2handle = {h.num: h for h in self.sems.allocated().values()}
        for w in waits:
            assert w.wait_mode == "sem-ge-imm", w
            h = id2handle.get(w.id)
            if h is not None:
                nc.sync.wait_ge(h, w.wait_value)
        nc.all_engine_barrier()
        popped = nc._tile_sem_poison_stack.pop()
        assert popped is self._sem_poison
        nc.clear_and_free_semaphores(list(self.sems.allocated().values()))
        nc.all_engine_barrier()

    TileContext._drain_and_barrier = _drain_and_barrier
    TileContext._drain_waits_patched = True

    # This walrus also rejects >1 sync wait on ordinary instructions.
    # Rewrite the BIR before compile: hoist excess waits onto standalone
    # single-wait EventSemaphore instructions on the same engine, placed
    # immediately before the owning instruction (same program order).
    import json as _json

    import concourse.bass2jax as _b2j
    import concourse.bass_utils as _bu

    def _split_bir_multiwaits(bir_json):
        j = _json.loads(bir_json)
        for f in j["functions"]:
            for bb in f["blocks"]:
                out = []
                for ins in bb["instructions"]:
                    si = ins.get("sync_info")
                    ow = (si or {}).get("on_wait") or []
                    if len(ow) > 1:
                        keep, hoist = [], []
                        for w in ow:
                            if w.get("wait_mode") == "sem-ge-imm":
                                hoist.append(w)
                            else:
                                keep.append(w)
                        if not keep and hoist:
                            keep.append(hoist.pop())
                        if len(keep) > 1:
                            raise RuntimeError(
                                f"can't split waits on {ins['name']}: {keep}"
                            )
                        for i, w in enumerate(hoist):
                            out.append(
                                {
                                    "debug": ins.get("debug", 0),
                                    "engine": ins["engine"],
                                    "ins": [],
                                    "outs": [],
                                    "name": f"{ins['name']}.hw{i}",
                                    "opcode": "EventSemaphore",
                                    "sync_info": {
                                        "on_update": [],
                                        "on_wait": [w],
                                    },
                                }
                            )
                        si["on_wait"] = keep
                    out.append(ins)
                bb["instructions"] = out
        return _json.dumps(j).encode()

    _orig_cbk = _bu.compile_bir_kernel

    def _cbk(bir_json, tmpdir, neff_name="file.neff"):
        return _orig_cbk(_split_bir_multiwaits(bir_json), tmpdir, neff_name)

    _bu.compile_bir_kernel = _cbk
    _b2j.compile_bir_kernel = _cbk


def build_nc(S):
    from contextlib import ExitStack

    import concourse.bass as bass
    import concourse.mybir as mybir
    from concourse.tile import TileContext

    _patch_tile_drain()

    f32 = mybir.dt.float32
    bf = mybir.dt.bfloat16

    CHUNK = S // NCORES    # output rows per core
    NST = CHUNK // P       # seq tiles per core in o_proj
    NHC = HID // P         # hidden chunks
    NKT = S // P           # key tiles
    NQC = S // QCHUNK      # attention q chunks
    NQTR = S // QS         # qkv-phase seq chunks
    NQ4 = HID // HQ        # o_proj hid quarters

    nc = bass.Bass(num_devices=NCORES)
    xT = nc.declare_dram_parameter("xT", [HID, S], bf, isOutput=False)
    wq = nc.declare_dram_parameter("wq", [HID, ROWS_Q], bf, isOutput=False)
    wk = nc.declare_dram_parameter("wk", [HID, D], bf, isOutput=False)
    wv = nc.declare_dram_parameter("wv", [HID, D], bf, isOutput=False)
    wo = nc.declare_dram_parameter("wo", [HID, HID], bf, isOutput=False)
    cosq = nc.declare_dram_parameter("cosq", [D, S], bf, isOutput=False)
    sinqm = nc.declare_dram_parameter("sinqm", [D, S], bf, isOutput=False)
    cosk = nc.declare_dram_parameter("cosk", [D, S], bf, isOutput=False)
    sinkm = nc.declare_dram_parameter("sinkm", [D, S], bf, isOutput=False)
    maskT = nc.declare_dram_parameter("maskT", [P, 512], bf, isOutput=False)
    ident = nc.declare_dram_parameter("ident", [P, P], bf, isOutput=False)
    out = nc.declare_dram_parameter("out", [CHUNK, HID], f32, isOutput=True)

    # one AllToAll per head: the collective cores serialize, and phase 2
    # is ACT-bound, so four small pipelined collectives overlap the
    # attention tail and land each head's data as early as possible.
    a2a_send = [
        nc.dram_tensor(f"a2a_send{h}", [NCORES, D, CHUNK], bf) for h in range(HPC)
    ]
    a2a_recv = [
        nc.dram_tensor(f"a2a_recv{h}", [NCORES, D, CHUNK], bf) for h in range(HPC)
    ]

    with TileContext(nc, num_cores=NCORES) as tc, ExitStack() as top:
        consts = top.enter_context(tc.tile_pool(name="consts", bufs=1))
        persist = top.enter_context(tc.tile_pool(name="persist", bufs=1))
        # Wo stream pool lives at top level so its SBUF region never
        # overlaps a released pool (which would gate its DMAs on phase-1
        # teardown); ring of 18 tiles = 4.5 MB of early prefetch.
        wo_pool = top.enter_context(tc.tile_pool(name="wo_pool", bufs=18))

        # const tiles created here; their DMAs are emitted inside phase 1
        # after the first matmul operands, so startup isn't DMA-gated.
        cosq_sb = consts.tile([D, S], bf, name="cosq_sb")
        sinqm_sb = consts.tile([D, S], bf, name="sinqm_sb")
        cosk_sb = consts.tile([D, S], bf, name="cosk_sb")
        sinkm_sb = consts.tile([D, S], bf, name="sinkm_sb")
        maskT_sb = consts.tile([P, 512], bf, name="maskT_sb")
        ident_sb = consts.tile([P, P], bf, name="ident_sb")

        qT_sb = [persist.tile([D, S], bf, name=f"qT{h}") for h in range(HPC)]
        kT_sb = persist.tile([D, S], bf, name="kT_sb")
        vnat = persist.tile([P, NKT, D + 1], bf, name="vnat")
        nc.gpsimd.memset(vnat[:, :, D : D + 1], 1.0)

        # ---- phase 1: qkv projections + rope ----
        _markers = []

        def _mark(name):
            _markers.append((name, len(nc.inst_map)))

        _mark("p1_qkv")
        with ExitStack() as ph1:
            w_pool = ph1.enter_context(tc.tile_pool(name="w_pool", bufs=1))
            tmp_pool = ph1.enter_context(tc.tile_pool(name="tmp_pool", bufs=2))
            acc_pool = ph1.enter_context(
                tc.tile_pool(name="acc_pool", bufs=7, space="PSUM")
            )
            vtr_pool = ph1.enter_context(
                tc.tile_pool(name="vtr_pool", bufs=1, space="PSUM")
            )

            xq_pool = ph1.enter_context(tc.tile_pool(name="xq_pool", bufs=2))
            wk_all = w_pool.tile([P, NHC, D], bf, name="wk_all")
            wv_all = w_pool.tile([P, NHC, D], bf, name="wv_all")
            wq_all = w_pool.tile([P, NHC, ROWS_Q], bf, name="wq_all")
            vT_sb = w_pool.tile([D, S], bf, name="vT_sb")
            xq = xq_pool.tile([P, NHC, QS], bf, tag="xq", name="xq0")

            xT_r = xT.ap().rearrange("(a p) s -> p a s", p=P)
            wk_r = wk.ap().rearrange("(a p) c -> p a c", p=P)
            wv_r = wv.ap().rearrange("(a p) c -> p a c", p=P)
            wq_r = wq.ap().rearrange("(a p) c -> p a c", p=P)

            # DMA emission order == need order (chunk-major: every job's
            # chunk-c weights before xq chunk c+1).
            NC4 = NHC // 4
            nc.sync.dma_start(out=xq[:, 0:NC4, :], in_=xT_r[:, 0:NC4, 0:QS])
            nc.sync.dma_start(out=wk_all[:, 0:NC4, :], in_=wk_r[:, 0:NC4, :])
            nc.sync.dma_start(out=wv_all[:, 0:NC4, :], in_=wv_r[:, 0:NC4, :])
            for h in range(HPC):
                nc.sync.dma_start(
                    out=wq_all[:, 0:NC4, h * D : (h + 1) * D],
                    in_=wq_r[:, 0:NC4, h * D : (h + 1) * D],
                )
            for c in range(1, 4):
                hsl = slice(c * NC4, (c + 1) * NC4)
                nc.sync.dma_start(out=xq[:, hsl, :], in_=xT_r[:, hsl, 0:QS])
                nc.sync.dma_start(out=wk_all[:, hsl, :], in_=wk_r[:, hsl, :])
                nc.sync.dma_start(out=wv_all[:, hsl, :], in_=wv_r[:, hsl, :])
                for h in range(HPC):
                    nc.sync.dma_start(
                        out=wq_all[:, hsl, h * D : (h + 1) * D],
                        in_=wq_r[:, hsl, h * D : (h + 1) * D],
                    )
            # qtr 1's xq chunks queue before the rope constants: the 2-buf
            # ring makes this safe (fresh buffer, no WAR), and it keeps the
            # qtr0->qtr1 transition fed; later quarters are emitted one
            # quarter ahead inside the loop.
            xq_tiles = {0: xq}

            def emit_xq(qtr):
                t = xq_pool.tile([P, NHC, QS], bf, tag="xq", name=f"xq{qtr}")
                xq_tiles[qtr] = t
                qsl = slice(qtr * QS, (qtr + 1) * QS)
                for c in range(4):
                    hsl = slice(c * NC4, (c + 1) * NC4)
                    nc.sync.dma_start(out=t[:, hsl, :], in_=xT_r[:, hsl, qsl])

            emit_xq(1)
            nc.sync.dma_start(out=cosk_sb, in_=cosk[:, :])
            nc.sync.dma_start(out=sinkm_sb, in_=sinkm[:, :])
            nc.sync.dma_start(out=cosq_sb, in_=cosq[:, :])
            nc.sync.dma_start(out=sinqm_sb, in_=sinqm[:, :])
            nc.sync.dma_start(out=ident_sb, in_=ident[:, :])
            nc.sync.dma_start(out=maskT_sb, in_=maskT[:, :])

            # chunk-major: all 6 projection jobs accumulate together while
            # streaming xq one hidden-chunk at a time, so a 1 MB chunk DMA
            # feeds 6x8 matmuls (10 us) instead of 8 (1.7 us) -> the xq
            # stream never starves the PE even with a single xq buffer.
            jobs = [("k", 0), ("v", 0)] + [("q", h) for h in range(HPC)]

            def emit_finish(qtr, kind, h, acc):
                sl = slice(qtr * QS, (qtr + 1) * QS)
                if kind == "v":
                    nc.scalar.copy(out=vT_sb[:, sl], in_=acc)
                    for t in range(QS // P):
                        kt = qtr * (QS // P) + t
                        vtr = vtr_pool.tile(
                            [P, P], bf, tag="vtr", name=f"vtr{kt}"
                        )
                        nc.tensor.transpose(
                            vtr, vT_sb[:, kt * P : (kt + 1) * P], ident_sb
                        )
                        nc.scalar.copy(out=vnat[:, kt, 0:D], in_=vtr)
                    return
                if kind == "q":
                    cos_t, sinm_t, dest = cosq_sb, sinqm_sb, qT_sb[h]
                else:
                    cos_t, sinm_t, dest = cosk_sb, sinkm_sb, kT_sb
                # rope: dest = acc*cos + rotate_half(acc)*sin, with the
                # sign of rotate_half folded into sinm (upper half of
                # sinm is negated); rotate is a partition-half swap.
                tcos = tmp_pool.tile(
                    [P, QS], bf, tag="tcos", name=f"tcos_{qtr}_{kind}{h}"
                )
                nc.vector.tensor_mul(tcos, acc, cos_t[:, sl])
                trot = tmp_pool.tile(
                    [P, QS], bf, tag="trot", name=f"trot_{qtr}_{kind}{h}"
                )
                nc.vector.tensor_mul(
                    trot[0:HD2, :], acc[HD2:D, :], sinm_t[0:HD2, sl]
                )
                nc.vector.tensor_mul(
                    trot[HD2:D, :], acc[0:HD2, :], sinm_t[HD2:D, sl]
                )
                nc.gpsimd.tensor_add(dest[:, sl], tcos, trot)

            for qtr in range(NQTR):
                sl = slice(qtr * QS, (qtr + 1) * QS)
                if qtr + 1 < NQTR and qtr + 1 not in xq_tiles:
                    emit_xq(qtr + 1)
                xq = xq_tiles[qtr]

                def lhsT_of(kind, h, hc):
                    if kind == "q":
                        return wq_all[:, hc, h * D : (h + 1) * D]
                    if kind == "k":
                        return wk_all[:, hc, :]
                    return wv_all[:, hc, :]

                if qtr < NQTR - 1:
                    # chunk-major: all 6 jobs accumulate together while xq
                    # streams one hidden-chunk at a time (a 1 MB chunk DMA
                    # feeds 6x8 matmuls), so the stream can't starve the PE.
                    accs = {}
                    for kind, h in jobs:
                        accs[(kind, h)] = acc_pool.tile(
                            [P, QS], f32, tag="acc", name=f"acc_{qtr}_{kind}{h}"
                        )
                    for hc in range(NHC):
                        for kind, h in jobs:
                            nc.tensor.matmul(
                                accs[(kind, h)],
                                lhsT=lhsT_of(kind, h, hc),
                                rhs=xq[:, hc, :],
                                start=(hc == 0),
                                stop=(hc == NHC - 1),
                            )
                    # v first: its transposes (PE) land right after the
                    # Pool copy, then the rope chains drain on DVE/Pool.
                    for kind, h in [("k", 0), ("v", 0)] + [("q", x) for x in range(HPC)]:
                        emit_finish(qtr, kind, h, accs[(kind, h)])
                else:
                    # last quarter job-major: each job's rope follows its
                    # own matmul run, so phase 1's tail (and the PSUM
                    # teardown gating phase 2) isn't one big rope bunch.
                    for kind, h in jobs:
                        acc = acc_pool.tile(
                            [P, QS], f32, tag="acc", name=f"acc_{qtr}_{kind}{h}"
                        )
                        for hc in range(NHC):
                            nc.tensor.matmul(
                                acc,
                                lhsT=lhsT_of(kind, h, hc),
                                rhs=xq[:, hc, :],
                                start=(hc == 0),
                                stop=(hc == NHC - 1),
                            )
                        emit_finish(qtr, kind, h, acc)

        # ---- phase 2: attention (S_T layout, no-max softmax) ----
        _mark("p2_attn")
        # second Wo stream pool in the SBUF zone phase 1 releases: its
        # DMAs start right as phase 2 begins, adding 36 more tiles (9 MB)
        # of Wo prefetch so the o_proj stream never starves.
        ph23 = ExitStack()
        wo_pool2 = ph23.enter_context(tc.tile_pool(name="wo_pool2", bufs=44))
        with ExitStack() as ph2:
            pt_pool = ph2.enter_context(tc.tile_pool(name="pt_pool", bufs=4))
            ob_pool = ph2.enter_context(tc.tile_pool(name="ob_pool", bufs=2))
            stage_pool = ph2.enter_context(tc.tile_pool(name="stage_pool", bufs=2))
            r_pool = ph2.enter_context(tc.tile_pool(name="r_pool", bufs=2))
            sp_pool = ph2.enter_context(
                tc.tile_pool(name="sp_pool", bufs=3, space="PSUM")
            )
            outp_pool = ph2.enter_context(
                tc.tile_pool(name="outp_pool", bufs=1, space="PSUM")
            )
            trp_pool = ph2.enter_context(
                tc.tile_pool(name="trp_pool", bufs=1, space="PSUM")
            )

            pending = []
            stage = None
            for h in range(HPC):
                stage = stage_pool.tile(
                    [P, NCORES, CHUNK], bf, tag="stage", name=f"stage{h}"
                )
                for qc in range(NQC):
                    nkt = (qc + 1) * (QCHUNK // P)
                    q0 = qc * QCHUNK
                    outps = None

                    # software-pipelined emission: scores run two key-tiles
                    # ahead of the exp-gated AV groups so the PE never
                    # idles on ACT; each outp's normalization piece is
                    # emitted right after its stop-kt so the trp transpose
                    # overlaps the remaining AV groups.
                    sps = {}

                    def emit_score(kt):
                        j = kt - 4 * qc
                        lo = 128 * j if j >= 0 else 0
                        sp = sp_pool.tile(
                            [P, QCHUNK], f32, tag="sp", name=f"sp_{h}_{qc}_{kt}"
                        )
                        nc.tensor.matmul(
                            sp[:, lo:QCHUNK],
                            lhsT=kT_sb[:, kt * P : (kt + 1) * P],
                            rhs=qT_sb[h][:, q0 + lo : q0 + QCHUNK],
                            start=True,
                            stop=True,
                        )
                        sps[kt] = (sp, lo, j)

                    def emit_av(kt):
                        sp, lo, j = sps.pop(kt)
                        pt = pt_pool.tile(
                            [P, QCHUNK], bf, tag="pt", name=f"pt_{h}_{qc}_{kt}"
                        )
                        nc.scalar.activation(
                            pt[:, lo:QCHUNK],
                            sp[:, lo:QCHUNK],
                            mybir.ActivationFunctionType.Exp,
                        )
                        if j >= 0:
                            # only the diagonal 128-block needs masking
                            # (columns beyond it have c >= 128 > any k);
                            # keeping the mask write narrow lets the
                            # j4 > j AV matmuls skip the mask dependency.
                            nc.gpsimd.tensor_mul(
                                pt[:, lo : lo + P],
                                pt[:, lo : lo + P],
                                maskT_sb[:, 0:P],
                            )
                        # mask-dependent diagonal block last for max slack
                        if j >= 0:
                            order = list(range(j + 1, 4)) + [j]
                        else:
                            order = list(range(4))
                        for j4 in order:
                            nc.tensor.matmul(
                                outps[j4],
                                lhsT=pt[:, j4 * P : (j4 + 1) * P],
                                rhs=vnat[:, kt, :],
                                start=(kt == 0),
                                stop=(kt == 4 * qc + j4),
                            )

                    def emit_norm_piece(j4, outps, h=h, qc=qc):
                        qt = qc * 4 + j4
                        r = r_pool.tile([P, 1], f32, tag="r", name=f"r_{h}_{qt}")
                        nc.vector.reciprocal(r, outps[j4][:, D : D + 1])
                        ob = ob_pool.tile([P, D], bf, tag="ob", name=f"ob_{h}_{qt}")
                        nc.vector.tensor_scalar_mul(ob, outps[j4][:, 0:D], r)
                        trp = trp_pool.tile(
                            [P, P], bf, tag="trp", name=f"trp_{h}_{qt}"
                        )
                        nc.tensor.transpose(trp, ob, ident_sb)
                        core_j, col = divmod(qt, NST)
                        nc.vector.tensor_copy(
                            out=stage[:, core_j, col * P : (col + 1) * P],
                            in_=trp,
                        )

                    for kt0 in range(min(2, nkt)):
                        emit_score(kt0)
                    # drain the previous qc's trailing norm pieces BEFORE
                    # allocating this qc's outp ring instances (a read of
                    # the old instance emitted after the new allocation
                    # races the new accumulation)
                    while pending:
                        pending.pop(0)()
                    # one PSUM bank per AV accumulator: accumulation
                    # groups are tracked per bank ("zero region"), so two
                    # groups cannot share one
                    outps = [
                        outp_pool.tile(
                            [P, 512], f32, tag=f"outp{j}", name=f"outp_{h}_{qc}_{j}"
                        )[:, 0 : D + 1]
                        for j in range(4)
                    ]
                    for kt in range(nkt):
                        if kt + 2 < nkt:
                            emit_score(kt + 2)
                        emit_av(kt)
                        # norm piece delayed two kts past its stop so the
                        # DVE recip/ob chain finishes before the PE hits
                        # the trp transpose
                        if kt - 2 >= 4 * qc:
                            emit_norm_piece(kt - 2 - 4 * qc, outps=outps)
                    # trailing pieces deferred into the next qc's stream
                    # (flushed before the collective at head end); bind
                    # this qc's outps/h/qc explicitly -- the enclosing
                    # variables are rebound by the next iteration
                    pending.append(
                        lambda f=emit_norm_piece, o=outps, hh=h, qq=qc: f(
                            2, outps=o, h=hh, qc=qq
                        )
                    )
                    pending.append(
                        lambda f=emit_norm_piece, o=outps, hh=h, qq=qc: f(
                            3, outps=o, h=hh, qc=qq
                        )
                    )
                while pending:
                    pending.pop(0)()
                # head fully staged: ONE strided send DMA (the DMA engine
                # reorders [d, m, s] -> [m, d, s]), then the AllToAll
                # (walrus only accepts collectives on the gpsimd engine).
                nc.sync.dma_start(
                    out=a2a_send[h].ap().rearrange("m d s -> d m s"),
                    in_=stage,
                )
                nc.gpsimd.collective_compute(
                    "AllToAll",
                    mybir.AluOpType.bypass,
                    replica_groups=[list(range(NCORES))],
                    ins=[a2a_send[h][:, :, :]],
                    outs=[a2a_recv[h][:, :, :]],
                )

        # ---- phase 3: o_proj on this core's sequence chunk ----
        # hid-quarter outer loop (2 seq tiles x 1024 f32 = 4 banks per
        # quarter, double-buffered = 8 banks); h-major accumulation inside
        # so quarter 0 starts as soon as collective 0 lands.
        _mark("p3_oproj")
        with ExitStack() as ph3:
            att_pool = ph3.enter_context(tc.tile_pool(name="att_pool", bufs=1))
            osb_pool = ph3.enter_context(tc.tile_pool(name="osb_pool", bufs=2))
            o_psum = ph3.enter_context(
                tc.tile_pool(name="o_psum", bufs=2, space="PSUM")
            )

            # per-head attT tiles (separate tiles so o_proj's deps are
            # exact). Pair B's recv DMAs are emitted only after pair A's
            # o_proj parts, so the wo stream's DMA queues aren't blocked
            # behind DMAs gated on the last collective.
            att_h = [
                att_pool.tile([P, NCORES, CHUNK], bf, name=f"att_h{h}")
                for h in range(HPC)
            ]

            def emit_recv(hs):
                for h in hs:
                    nc.sync.dma_start(
                        out=att_h[h],
                        in_=a2a_recv[h].ap().rearrange("m d s -> d m s"),
                    )

            emit_recv([0, 1])

            # Quarters 0 and 1 defer their h=3 contributions: the last
            # head's AllToAll lands well after phase 2 ends, so h0-h2 of
            # two quarters accumulate first (both pos rings live = 8
            # banks), then the h3 parts close them out, then quarters
            # 2-3 run start-to-finish.
            wo_i = 0
            pos_by_q4 = {}

            def emit_part(q4, hs):
                h_sl = slice(q4 * HQ, (q4 + 1) * HQ)
                if q4 not in pos_by_q4:
                    pos_by_q4[q4] = [
                        o_psum.tile(
                            [P, HQ], f32, tag=f"po{st}", name=f"po_{q4}_{st}"
                        )
                        for st in range(NST)
                    ]
                pos = pos_by_q4[q4]
                nonlocal wo_i
                for h in hs:
                    for m in range(NCORES):
                        fc = 4 * m + h
                        # alternate tiles between the two stream pools:
                        # combined ring depth 18+44 = 62 tiles (15.5 MB)
                        pool_sel = wo_pool2 if wo_i % 62 < 44 else wo_pool
                        wo_i += 1
                        wo_sb = pool_sel.tile(
                            [P, HQ], bf, tag="wo_sb", name=f"wo_{q4}_{fc}"
                        )
                        nc.sync.dma_start(
                            out=wo_sb, in_=wo[fc * P : (fc + 1) * P, h_sl]
                        )
                        first = h == 0 and m == 0
                        last = h == HPC - 1 and m == NCORES - 1
                        for st in range(NST):
                            for s4 in range(HQ // 512):
                                nc.tensor.matmul(
                                    pos[st][:, s4 * 512 : (s4 + 1) * 512],
                                    lhsT=att_h[h][:, m, st * P : (st + 1) * P],
                                    rhs=wo_sb[:, s4 * 512 : (s4 + 1) * 512],
                                    start=first,
                                    stop=last,
                                )
                if hs[-1] == HPC - 1:
                    for st in range(NST):
                        osb = osb_pool.tile(
                            [P, HQ], f32, tag="osb", name=f"osb_{q4}_{st}"
                        )
                        nc.vector.tensor_copy(out=osb, in_=pos[st])
                        nc.sync.dma_start(
                            out=out[st * P : (st + 1) * P, h_sl], in_=osb
                        )
                    del pos_by_q4[q4]

            emit_part(0, [0, 1])
            emit_part(1, [0, 1])
            emit_recv([2])
            emit_part(0, [2])
            emit_part(1, [2])
            emit_recv([3])
            emit_part(0, [3])
            emit_part(1, [3])
            emit_part(2, [0, 1, 2, 3])
            emit_part(3, [0, 1, 2, 3])
        ph23.close()

    _mark("end")
    global _PHASE_MARKERS
    _PHASE_MARKERS = [
        (n, lo, hi)
        for (n, lo), (_, hi) in zip(_markers, _markers[1:])
    ]
    return nc


def make_in_maps(x, Wq, Wk, Wv, Wo):
    S = x.shape[1]
    xT = np.ascontiguousarray(x.reshape(S, HID).T.astype(np.float32)).astype(BF)
    woT = np.ascontiguousarray(Wo.astype(np.float32).T).astype(BF)

    inv_freq = 1.0 / (
        ROPE_THETA ** (np.arange(0, D, 2, dtype=np.float32) / np.float32(D))
    )
    t = np.arange(S, dtype=np.float32)
    freqs = np.outer(t, inv_freq).astype(np.float32)
    emb = np.concatenate([freqs, freqs], axis=1)
    cosT = np.cos(emb).T.astype(np.float32)  # [D, S]
    sinT = np.sin(emb).T.astype(np.float32)
    # sign-folded sin for the partition-shifted rotate_half:
    #   trot[0:64]  = acc[64:128] * (-sin[0:64])
    #   trot[64:128] = acc[0:64]  *   sin[64:128]
    sinM = np.concatenate([-sinT[0:HD2], sinT[HD2:D]], axis=0)
    scale = np.float32(1.0 / np.sqrt(np.float32(D)))
    cosq = np.ascontiguousarray(cosT * scale).astype(BF)
    sinqm = np.ascontiguousarray(sinM * scale).astype(BF)
    cosk = np.ascontiguousarray(cosT).astype(BF)
    sinkm = np.ascontiguousarray(sinM).astype(BF)

    mask = np.zeros((P, 512), dtype=np.float32)
    for k in range(P):
        mask[k, k:] = 1.0
    maskT = mask.astype(BF)
    ident = np.eye(P, dtype=np.float32).astype(BF)

    in_maps = []
    for m in range(NCORES):
        wqT = np.ascontiguousarray(
            Wq[m * ROWS_Q : (m + 1) * ROWS_Q, :].astype(np.float32).T
        ).astype(BF)
        wkT = np.ascontiguousarray(
            Wk[m * D : (m + 1) * D, :].astype(np.float32).T
        ).astype(BF)
        wvT = np.ascontiguousarray(
            Wv[m * D : (m + 1) * D, :].astype(np.float32).T
        ).astype(BF)
        in_maps.append(
            dict(
                xT=xT,
                wq=wqT,
                wk=wkT,
                wv=wvT,
                wo=woT,
                cosq=cosq,
                sinqm=sinqm,
                cosk=cosk,
                sinkm=sinkm,
                maskT=maskT,
                ident=ident,
            )
        )
    return in_maps


def gather_out(results, S):
    parts = [np.asarray(results[c]["out"], dtype=np.float32) for c in range(NCORES)]
    return np.concatenate(parts, axis=0).reshape(1, S, HID)


def kernel(x, Wq, Wk, Wv, Wo):
    from concourse.bass_utils import run_bass_kernel_spmd

    x = np.asarray(x)
    S = x.shape[1]
    nc = build_nc(S)
    in_maps = make_in_maps(x, np.asarray(Wq), np.asarray(Wk), np.asarray(Wv), np.asarray(Wo))
    res = run_bass_kernel_spmd(nc, in_maps, list(range(NCORES)))
    return gather_out(res.results, S)


# revision 39
# speedup vs baseline: 2.1485x; 2.1485x over previous
"""Tensor-parallel Llama attention (GQA) on 8 TRN2 NeuronCores.

Strategy (v2):
  - Head-sharded QKV + attention: core m computes Q heads [4m, 4m+4) and
    KV head m (GQA group is exactly per-core, so no KV duplication).
  - All matmuls in bf16 with f32 PSUM accumulation (fp8 busts the 2e-2
    error budget: measured 2.6-4.5% max-rel error per fp8 stage).
  - Transposed [feature, seq] layout keeps the PE contraction dim natural.
  - RoPE rotate_half via partition-shifted multiplies with sign-folded
    sin constants; DVE reads the projection PSUM directly (no PE rotate
    matmul, no staging copy). The all-SBUF combine add runs on Pool.
  - Phase 1 is chunk-major: all 6 projection jobs accumulate in 6 PSUM
    banks while x streams through one double-buffered tile, so a 1 MB
    chunk DMA feeds ~10 us of matmuls (last quarter job-major so its
    rope tail doesn't gate the phase-2 PSUM handover).
  - Softmax without max-subtraction (scores O(17) << f32 exp overflow);
    row sums from an appended ones-column on V. Phase 2 is ACT-bound
    (exp), so the PE emission runs scores two key-tiles ahead of the
    exp-gated AV groups, and per-output normalization pieces trail two
    key-tiles behind their accumulator's stop.
  - Causal trimming: diagonal key-tiles compute/exp/AV only the valid
    query range; only the 128-wide diagonal block is masked (on Pool),
    and its AV matmul is ordered last.
  - Per-head staged AllToAll: normalized outputs transpose (PE) into a
    [P, NCORES, CHUNK] staging tile, ONE strided send DMA reorders it
    into the collective layout, the collective issues from gpsimd (the
    only engine walrus accepts), and ONE strided recv DMA lands each
    head. Four small collectives pipeline on the serialized collective
    cores and land each head as early as possible.
  - o_proj in 4 hid-quarters (2 seq tiles x 2 banks, double-buffered =
    8 banks), h-major inside so quarters 0/1 defer their late-head
    contributions past the collective landings; Wo streams through
    18+44-deep tile rings (the second ring reuses phase-1's SBUF).
  - Host gathers by concatenating the 8 [S/8, HID] outputs.

Hardware constraints baked in: gpsimd/Pool cannot access PSUM; PSUM
accumulation groups are bank-granular (no two groups per bank); PSUM
tile pools allocate whole banks; collectives only on gpsimd.
"""

import numpy as np
import ml_dtypes

H, KV, D, HID = 32, 8, 128, 4096
NCORES = 8
HPC = H // NCORES          # q heads per core
ROWS_Q = HPC * D           # q projection rows per core
P = 128
HD2 = D // 2
QCHUNK = 512               # attention q-chunk (score matmul free dim)
QS = 512                   # qkv-phase seq chunk
HQ = 1024                  # o_proj hid quarter width
ROPE_THETA = 10000.0
BF = ml_dtypes.bfloat16


def _patch_tile_drain():
    """This container's walrus build rejects a Drain instruction carrying
    semaphore waits ("Too many sync wait commands"). Re-emit the Tile tail
    drain's waits as standalone single-wait SP instructions, which the
    same walrus accepts, followed by a wait-free drain."""
    from concourse.tile import TileContext
    from concourse.vector_clock import ScopedClock

    if getattr(TileContext, "_drain_waits_patched", False):
        return

    def _drain_and_barrier(self, tick_clock, wait_clock):
        nc = self.nc
        probe = nc.sync.drain()
        wait_clock.add_sem_waits(
            probe.ins, ScopedClock({None: tick_clock.global_clock})
        )
        waits = list(probe.ins.sync_info.on_wait)
        probe.ins.sync_info.on_wait = []
        id2handle = {h.num: h for h in self.sems.allocated().values()}
        for w in waits:
            assert w.wait_mode == "sem-ge-imm", w
            h = id2handle.get(w.id)
            if h is not None:
                nc.sync.wait_ge(h, w.wait_value)
        nc.all_engine_barrier()
        popped = nc._tile_sem_poison_stack.pop()
        assert popped is self._sem_poison
        nc.clear_and_free_semaphores(list(self.sems.allocated().values()))
        nc.all_engine_barrier()

    TileContext._drain_and_barrier = _drain_and_barrier
    TileContext._drain_waits_patched = True

    # This walrus also rejects >1 sync wait on ordinary instructions.
    # Rewrite the BIR before compile: hoist excess waits onto standalone
    # single-wait EventSemaphore instructions on the same engine, placed
    # immediately before the owning instruction (same program order).
    import json as _json

    import concourse.bass2jax as _b2j
    import concourse.bass_utils as _bu

    def _split_bir_multiwaits(bir_json):
        j = _json.loads(bir_json)
        for f in j["functions"]:
            for bb in f["blocks"]:
                out = []
                for ins in bb["instructions"]:
                    si = ins.get("sync_info")
                    ow = (si or {}).get("on_wait") or []
                    if len(ow) > 1:
                        keep, hoist = [], []
                        for w in ow:
                            if w.get("wait_mode") == "sem-ge-imm":
                                hoist.append(w)
                            else:
                                keep.append(w)
                        if not keep and hoist:
                            keep.append(hoist.pop())
                        if len(keep) > 1:
                            raise RuntimeError(
                                f"can't split waits on {ins['name']}: {keep}"
                            )
                        for i, w in enumerate(hoist):
                            out.append(
                                {
                                    "debug": ins.get("debug", 0),
                                    "engine": ins["engine"],
                                    "ins": [],
                                    "outs": [],
                                    "name": f"{ins['name']}.hw{i}",
                                    "opcode": "EventSemaphore",
                                    "sync_info": {
                                        "on_update": [],
                                        "on_wait": [w],
                                    },
                                }
                            )
                        si["on_wait"] = keep
                    out.append(ins)
                bb["instructions"] = out
        return _json.dumps(j).encode()

    _orig_cbk = _bu.compile_bir_kernel

    def _cbk(bir_json, tmpdir, neff_name="file.neff"):
        return _orig_cbk(_split_bir_multiwaits(bir_json), tmpdir, neff_name)

    _bu.compile_bir_kernel = _cbk
    _b2j.compile_bir_kernel = _cbk


def build_nc(S):
    from contextlib import ExitStack

    import concourse.bass as bass
    import concourse.mybir as mybir
    from concourse.tile import TileContext

    _patch_tile_drain()

    f32 = mybir.dt.float32
    bf = mybir.dt.bfloat16

    CHUNK = S // NCORES    # output rows per core
    NST = CHUNK // P       # seq tiles per core in o_proj
    NHC = HID // P         # hidden chunks
    NKT = S // P           # key tiles
    NQC = S // QCHUNK      # attention q chunks
    NQTR = S // QS         # qkv-phase seq chunks
    NQ4 = HID // HQ        # o_proj hid quarters

    nc = bass.Bass(num_devices=NCORES)
    xT = nc.declare_dram_parameter("xT", [HID, S], bf, isOutput=False)
    wq = nc.declare_dram_parameter("wq", [HID, ROWS_Q], bf, isOutput=False)
    wk = nc.declare_dram_parameter("wk", [HID, D], bf, isOutput=False)
    wv = nc.declare_dram_parameter("wv", [HID, D], bf, isOutput=False)
    wo = nc.declare_dram_parameter("wo", [HID, HID], bf, isOutput=False)
    cosq = nc.declare_dram_parameter("cosq", [D, S], bf, isOutput=False)
    sinqm = nc.declare_dram_parameter("sinqm", [D, S], bf, isOutput=False)
    cosk = nc.declare_dram_parameter("cosk", [D, S], bf, isOutput=False)
    sinkm = nc.declare_dram_parameter("sinkm", [D, S], bf, isOutput=False)
    maskT = nc.declare_dram_parameter("maskT", [P, 512], bf, isOutput=False)
    ident = nc.declare_dram_parameter("ident", [P, P], bf, isOutput=False)
    out = nc.declare_dram_parameter("out", [CHUNK, HID], f32, isOutput=True)

    # one AllToAll per head: the collective cores serialize, and phase 2
    # is ACT-bound, so four small pipelined collectives overlap the
    # attention tail and land each head's data as early as possible.
    a2a_send = [
        nc.dram_tensor(f"a2a_send{h}", [NCORES, D, CHUNK], bf) for h in range(HPC)
    ]
    a2a_recv = [
        nc.dram_tensor(f"a2a_recv{h}", [NCORES, D, CHUNK], bf) for h in range(HPC)
    ]

    with TileContext(nc, num_cores=NCORES) as tc, ExitStack() as top:
        consts = top.enter_context(tc.tile_pool(name="consts", bufs=1))
        persist = top.enter_context(tc.tile_pool(name="persist", bufs=1))
        # Wo stream pool lives at top level so its SBUF region never
        # overlaps a released pool (which would gate its DMAs on phase-1
        # teardown); ring of 18 tiles = 4.5 MB of early prefetch.
        wo_pool = top.enter_context(tc.tile_pool(name="wo_pool", bufs=18))

        # const tiles created here; their DMAs are emitted inside phase 1
        # after the first matmul operands, so startup isn't DMA-gated.
        cosq_sb = consts.tile([D, S], bf, name="cosq_sb")
        sinqm_sb = consts.tile([D, S], bf, name="sinqm_sb")
        cosk_sb = consts.tile([D, S], bf, name="cosk_sb")
        sinkm_sb = consts.tile([D, S], bf, name="sinkm_sb")
        maskT_sb = consts.tile([P, 512], bf, name="maskT_sb")
        ident_sb = consts.tile([P, P], bf, name="ident_sb")

        qT_sb = [persist.tile([D, S], bf, name=f"qT{h}") for h in range(HPC)]
        kT_sb = persist.tile([D, S], bf, name="kT_sb")
        vnat = persist.tile([P, NKT, D + 1], bf, name="vnat")
        nc.gpsimd.memset(vnat[:, :, D : D + 1], 1.0)

        # ---- phase 1: qkv projections + rope ----
        _markers = []

        def _mark(name):
            _markers.append((name, len(nc.inst_map)))

        _mark("p1_qkv")
        with ExitStack() as ph1:
            w_pool = ph1.enter_context(tc.tile_pool(name="w_pool", bufs=1))
            tmp_pool = ph1.enter_context(tc.tile_pool(name="tmp_pool", bufs=2))
            acc_pool = ph1.enter_context(
                tc.tile_pool(name="acc_pool", bufs=7, space="PSUM")
            )
            vtr_pool = ph1.enter_context(
                tc.tile_pool(name="vtr_pool", bufs=1, space="PSUM")
            )

            xq_pool = ph1.enter_context(tc.tile_pool(name="xq_pool", bufs=2))
            wk_all = w_pool.tile([P, NHC, D], bf, name="wk_all")
            wv_all = w_pool.tile([P, NHC, D], bf, name="wv_all")
            wq_all = w_pool.tile([P, NHC, ROWS_Q], bf, name="wq_all")
            vT_sb = w_pool.tile([D, S], bf, name="vT_sb")
            xq = xq_pool.tile([P, NHC, QS], bf, tag="xq", name="xq0")

            xT_r = xT.ap().rearrange("(a p) s -> p a s", p=P)
            wk_r = wk.ap().rearrange("(a p) c -> p a c", p=P)
            wv_r = wv.ap().rearrange("(a p) c -> p a c", p=P)
            wq_r = wq.ap().rearrange("(a p) c -> p a c", p=P)

            # DMA emission order == need order (chunk-major: every job's
            # chunk-c weights before xq chunk c+1).
            NC4 = NHC // 4
            nc.sync.dma_start(out=xq[:, 0:NC4, :], in_=xT_r[:, 0:NC4, 0:QS])
            nc.sync.dma_start(out=wk_all[:, 0:NC4, :], in_=wk_r[:, 0:NC4, :])
            nc.sync.dma_start(out=wv_all[:, 0:NC4, :], in_=wv_r[:, 0:NC4, :])
            for h in range(HPC):
                nc.sync.dma_start(
                    out=wq_all[:, 0:NC4, h * D : (h + 1) * D],
                    in_=wq_r[:, 0:NC4, h * D : (h + 1) * D],
                )
            for c in range(1, 4):
                hsl = slice(c * NC4, (c + 1) * NC4)
                nc.sync.dma_start(out=xq[:, hsl, :], in_=xT_r[:, hsl, 0:QS])
                nc.sync.dma_start(out=wk_all[:, hsl, :], in_=wk_r[:, hsl, :])
                nc.sync.dma_start(out=wv_all[:, hsl, :], in_=wv_r[:, hsl, :])
                for h in range(HPC):
                    nc.sync.dma_start(
                        out=wq_all[:, hsl, h * D : (h + 1) * D],
                        in_=wq_r[:, hsl, h * D : (h + 1) * D],
                    )
            # qtr 1's xq chunks queue before the rope constants: the 2-buf
            # ring makes this safe (fresh buffer, no WAR), and it keeps the
            # qtr0->qtr1 transition fed; later quarters are emitted one
            # quarter ahead inside the loop.
            xq_tiles = {0: xq}

            def emit_xq(qtr):
                t = xq_pool.tile([P, NHC, QS], bf, tag="xq", name=f"xq{qtr}")
                xq_tiles[qtr] = t
                qsl = slice(qtr * QS, (qtr + 1) * QS)
                for c in range(4):
                    hsl = slice(c * NC4, (c + 1) * NC4)
                    nc.sync.dma_start(out=t[:, hsl, :], in_=xT_r[:, hsl, qsl])

            emit_xq(1)
            nc.sync.dma_start(out=cosk_sb, in_=cosk[:, :])
            nc.sync.dma_start(out=sinkm_sb, in_=sinkm[:, :])
            nc.sync.dma_start(out=cosq_sb, in_=cosq[:, :])
            nc.sync.dma_start(out=sinqm_sb, in_=sinqm[:, :])
            nc.sync.dma_start(out=ident_sb, in_=ident[:, :])
            nc.sync.dma_start(out=maskT_sb, in_=maskT[:, :])

            # chunk-major: all 6 projection jobs accumulate together while
            # streaming xq one hidden-chunk at a time, so a 1 MB chunk DMA
            # feeds 6x8 matmuls (10 us) instead of 8 (1.7 us) -> the xq
            # stream never starves the PE even with a single xq buffer.
            jobs = [("k", 0), ("v", 0)] + [("q", h) for h in range(HPC)]

            def emit_finish(qtr, kind, h, acc):
                sl = slice(qtr * QS, (qtr + 1) * QS)
                if kind == "v":
                    nc.scalar.copy(out=vT_sb[:, sl], in_=acc)
                    for t in range(QS // P):
                        kt = qtr * (QS // P) + t
                        vtr = vtr_pool.tile(
                            [P, P], bf, tag="vtr", name=f"vtr{kt}"
                        )
                        nc.tensor.transpose(
                            vtr, vT_sb[:, kt * P : (kt + 1) * P], ident_sb
                        )
                        nc.scalar.copy(out=vnat[:, kt, 0:D], in_=vtr)
                    return
                if kind == "q":
                    cos_t, sinm_t, dest = cosq_sb, sinqm_sb, qT_sb[h]
                else:
                    cos_t, sinm_t, dest = cosk_sb, sinkm_sb, kT_sb
                # rope: dest = acc*cos + rotate_half(acc)*sin, with the
                # sign of rotate_half folded into sinm (upper half of
                # sinm is negated); rotate is a partition-half swap.
                tcos = tmp_pool.tile(
                    [P, QS], bf, tag="tcos", name=f"tcos_{qtr}_{kind}{h}"
                )
                nc.vector.tensor_mul(tcos, acc, cos_t[:, sl])
                trot = tmp_pool.tile(
                    [P, QS], bf, tag="trot", name=f"trot_{qtr}_{kind}{h}"
                )
                nc.vector.tensor_mul(
                    trot[0:HD2, :], acc[HD2:D, :], sinm_t[0:HD2, sl]
                )
                nc.vector.tensor_mul(
                    trot[HD2:D, :], acc[0:HD2, :], sinm_t[HD2:D, sl]
                )
                nc.gpsimd.tensor_add(dest[:, sl], tcos, trot)

            for qtr in range(NQTR):
                sl = slice(qtr * QS, (qtr + 1) * QS)
                if qtr + 1 < NQTR and qtr + 1 not in xq_tiles:
                    emit_xq(qtr + 1)
                xq = xq_tiles[qtr]

                def lhsT_of(kind, h, hc):
                    if kind == "q":
                        return wq_all[:, hc, h * D : (h + 1) * D]
                    if kind == "k":
                        return wk_all[:, hc, :]
                    return wv_all[:, hc, :]

                if qtr < NQTR - 1:
                    # chunk-major: all 6 jobs accumulate together while xq
                    # streams one hidden-chunk at a time (a 1 MB chunk DMA
                    # feeds 6x8 matmuls), so the stream can't starve the PE.
                    accs = {}
                    for kind, h in jobs:
                        accs[(kind, h)] = acc_pool.tile(
                            [P, QS], f32, tag="acc", name=f"acc_{qtr}_{kind}{h}"
                        )
                    for hc in range(NHC):
                        for kind, h in jobs:
                            nc.tensor.matmul(
                                accs[(kind, h)],
                                lhsT=lhsT_of(kind, h, hc),
                                rhs=xq[:, hc, :],
                                start=(hc == 0),
                                stop=(hc == NHC - 1),
                            )
                    # v first: its transposes (PE) land right after the
                    # Pool copy, then the rope chains drain on DVE/Pool.
                    for kind, h in [("k", 0), ("v", 0)] + [("q", x) for x in range(HPC)]:
                        emit_finish(qtr, kind, h, accs[(kind, h)])
                else:
                    # last quarter job-major: each job's rope follows its
                    # own matmul run, so phase 1's tail (and the PSUM
                    # teardown gating phase 2) isn't one big rope bunch.
                    for kind, h in jobs:
                        acc = acc_pool.tile(
                            [P, QS], f32, tag="acc", name=f"acc_{qtr}_{kind}{h}"
                        )
                        for hc in range(NHC):
                            nc.tensor.matmul(
                                acc,
                                lhsT=lhsT_of(kind, h, hc),
                                rhs=xq[:, hc, :],
                                start=(hc == 0),
                                stop=(hc == NHC - 1),
                            )
                        emit_finish(qtr, kind, h, acc)

        # ---- phase 2: attention (S_T layout, no-max softmax) ----
        _mark("p2_attn")
        # second Wo stream pool in the SBUF zone phase 1 releases: its
        # DMAs start right as phase 2 begins, adding 36 more tiles (9 MB)
        # of Wo prefetch so the o_proj stream never starves.
        ph23 = ExitStack()
        wo_pool2 = ph23.enter_context(tc.tile_pool(name="wo_pool2", bufs=44))
        with ExitStack() as ph2:
            pt_pool = ph2.enter_context(tc.tile_pool(name="pt_pool", bufs=4))
            ob_pool = ph2.enter_context(tc.tile_pool(name="ob_pool", bufs=2))
            stage_pool = ph2.enter_context(tc.tile_pool(name="stage_pool", bufs=2))
            r_pool = ph2.enter_context(tc.tile_pool(name="r_pool", bufs=2))
            sp_pool = ph2.enter_context(
                tc.tile_pool(name="sp_pool", bufs=3, space="PSUM")
            )
            outp_pool = ph2.enter_context(
                tc.tile_pool(name="outp_pool", bufs=1, space="PSUM")
            )
            trp_pool = ph2.enter_context(
                tc.tile_pool(name="trp_pool", bufs=1, space="PSUM")
            )

            pending = []
            stage = None
            for h in range(HPC):
                stage = stage_pool.tile(
                    [P, NCORES, CHUNK], bf, tag="stage", name=f"stage{h}"
                )
                for qc in range(NQC):
                    nkt = (qc + 1) * (QCHUNK // P)
                    q0 = qc * QCHUNK
                    outps = None

                    # software-pipelined emission: scores run two key-tiles
                    # ahead of the exp-gated AV groups so the PE never
                    # idles on ACT; each outp's normalization piece is
                    # emitted right after its stop-kt so the trp transpose
                    # overlaps the remaining AV groups.
                    sps = {}

                    def emit_score(kt):
                        j = kt - 4 * qc
                        lo = 128 * j if j >= 0 else 0
                        sp = sp_pool.tile(
                            [P, QCHUNK], f32, tag="sp", name=f"sp_{h}_{qc}_{kt}"
                        )
                        nc.tensor.matmul(
                            sp[:, lo:QCHUNK],
                            lhsT=kT_sb[:, kt * P : (kt + 1) * P],
                            rhs=qT_sb[h][:, q0 + lo : q0 + QCHUNK],
                            start=True,
                            stop=True,
                        )
                        sps[kt] = (sp, lo, j)

                    def emit_av(kt):
                        sp, lo, j = sps.pop(kt)
                        pt = pt_pool.tile(
                            [P, QCHUNK], bf, tag="pt", name=f"pt_{h}_{qc}_{kt}"
                        )
                        nc.scalar.activation(
                            pt[:, lo:QCHUNK],
                            sp[:, lo:QCHUNK],
                            mybir.ActivationFunctionType.Exp,
                        )
                        if j >= 0:
                            # only the diagonal 128-block needs masking
                            # (columns beyond it have c >= 128 > any k);
                            # keeping the mask write narrow lets the
                            # j4 > j AV matmuls skip the mask dependency.
                            nc.gpsimd.tensor_mul(
                                pt[:, lo : lo + P],
                                pt[:, lo : lo + P],
                                maskT_sb[:, 0:P],
                            )
                        # mask-dependent diagonal block last for max slack
                        if j >= 0:
                            order = list(range(j + 1, 4)) + [j]
                        else:
                            order = list(range(4))
                        for j4 in order:
                            nc.tensor.matmul(
                                outps[j4],
                                lhsT=pt[:, j4 * P : (j4 + 1) * P],
                                rhs=vnat[:, kt, :],
                                start=(kt == 0),
                                stop=(kt == 4 * qc + j4),
                            )

                    def emit_norm_piece(j4, outps, h=h, qc=qc):
                        qt = qc * 4 + j4
                        r = r_pool.tile([P, 1], f32, tag="r", name=f"r_{h}_{qt}")
                        nc.vector.reciprocal(r, outps[j4][:, D : D + 1])
                        ob = ob_pool.tile([P, D], bf, tag="ob", name=f"ob_{h}_{qt}")
                        nc.vector.tensor_scalar_mul(ob, outps[j4][:, 0:D], r)
                        trp = trp_pool.tile(
                            [P, P], bf, tag="trp", name=f"trp_{h}_{qt}"
                        )
                        nc.tensor.transpose(trp, ob, ident_sb)
                        core_j, col = divmod(qt, NST)
                        nc.vector.tensor_copy(
                            out=stage[:, core_j, col * P : (col + 1) * P],
                            in_=trp,
                        )

                    for kt0 in range(min(2, nkt)):
                        emit_score(kt0)
                    # drain the previous qc's trailing norm pieces BEFORE
                    # allocating this qc's outp ring instances (a read of
                    # the old instance emitted after the new allocation
                    # races the new accumulation)
                    while pending:
                        pending.pop(0)()
                    # one PSUM bank per AV accumulator: accumulation
                    # groups are tracked per bank ("zero region"), so two
                    # groups cannot share one
                    outps = [
                        outp_pool.tile(
                            [P, 512], f32, tag=f"outp{j}", name=f"outp_{h}_{qc}_{j}"
                        )[:, 0 : D + 1]
                        for j in range(4)
                    ]
                    for kt in range(nkt):
                        if kt + 2 < nkt:
                            emit_score(kt + 2)
                        emit_av(kt)
                        # norm piece delayed two kts past its stop so the
                        # DVE recip/ob chain finishes before the PE hits
                        # the trp transpose
                        if kt - 2 >= 4 * qc:
                            emit_norm_piece(kt - 2 - 4 * qc, outps=outps)
                    # trailing pieces deferred into the next qc's stream
                    # (flushed before the collective at head end); bind
                    # this qc's outps/h/qc explicitly -- the enclosing
                    # variables are rebound by the next iteration
                    pending.append(
                        lambda f=emit_norm_piece, o=outps, hh=h, qq=qc: f(
                            2, outps=o, h=hh, qc=qq
                        )
                    )
                    pending.append(
                        lambda f=emit_norm_piece, o=outps, hh=h, qq=qc: f(
                            3, outps=o, h=hh, qc=qq
                        )
                    )
                while pending:
                    pending.pop(0)()
                # head fully staged: ONE strided send DMA (the DMA engine
                # reorders [d, m, s] -> [m, d, s]), then the AllToAll
                # (walrus only accepts collectives on the gpsimd engine).
                nc.sync.dma_start(
                    out=a2a_send[h].ap().rearrange("m d s -> d m s"),
                    in_=stage,
                )
                nc.gpsimd.collective_compute(
                    "AllToAll",
                    mybir.AluOpType.bypass,
                    replica_groups=[list(range(NCORES))],
                    ins=[a2a_send[h][:, :, :]],
                    outs=[a2a_recv[h][:, :, :]],
                )

        # ---- phase 3: o_proj on this core's sequence chunk ----
        # hid-quarter outer loop (2 seq tiles x 1024 f32 = 4 banks per
        # quarter, double-buffered = 8 banks); h-major accumulation inside
        # so quarter 0 starts as soon as collective 0 lands.
        _mark("p3_oproj")
        with ExitStack() as ph3:
            att_pool = ph3.enter_context(tc.tile_pool(name="att_pool", bufs=1))
            osb_pool = ph3.enter_context(tc.tile_pool(name="osb_pool", bufs=2))
            o_psum = ph3.enter_context(
                tc.tile_pool(name="o_psum", bufs=2, space="PSUM")
            )

            # per-head attT tiles (separate tiles so o_proj's deps are
            # exact). Pair B's recv DMAs are emitted only after pair A's
            # o_proj parts, so the wo stream's DMA queues aren't blocked
            # behind DMAs gated on the last collective.
            att_h = [
                att_pool.tile([P, NCORES, CHUNK], bf, name=f"att_h{h}")
                for h in range(HPC)
            ]

            def emit_recv(hs):
                for h in hs:
                    nc.sync.dma_start(
                        out=att_h[h],
                        in_=a2a_recv[h].ap().rearrange("m d s -> d m s"),
                    )

            emit_recv([0, 1])

            # Quarters 0 and 1 defer their h=3 contributions: the last
            # head's AllToAll lands well after phase 2 ends, so h0-h2 of
            # two quarters accumulate first (both pos rings live = 8
            # banks), then the h3 parts close them out, then quarters
            # 2-3 run start-to-finish.
            wo_i = 0
            pos_by_q4 = {}

            def emit_part(q4, hs):
                h_sl = slice(q4 * HQ, (q4 + 1) * HQ)
                if q4 not in pos_by_q4:
                    pos_by_q4[q4] = [
                        o_psum.tile(
                            [P, HQ], f32, tag=f"po{st}", name=f"po_{q4}_{st}"
                        )
                        for st in range(NST)
                    ]
                pos = pos_by_q4[q4]
                nonlocal wo_i
                for h in hs:
                    for m in range(NCORES):
                        fc = 4 * m + h
                        # alternate tiles between the two stream pools:
                        # combined ring depth 18+44 = 62 tiles (15.5 MB)
                        pool_sel = wo_pool2 if wo_i % 62 < 44 else wo_pool
                        wo_i += 1
                        wo_sb = pool_sel.tile(
                            [P, HQ], bf, tag="wo_sb", name=f"wo_{q4}_{fc}"
                        )
                        nc.sync.dma_start(
                            out=wo_sb, in_=wo[fc * P : (fc + 1) * P, h_sl]
                        )
                        first = h == 0 and m == 0
                        last = h == HPC - 1 and m == NCORES - 1
                        for st in range(NST):
                            for s4 in range(HQ // 512):
                                nc.tensor.matmul(
                                    pos[st][:, s4 * 512 : (s4 + 1) * 512],
                                    lhsT=att_h[h][:, m, st * P : (st + 1) * P],
                                    rhs=wo_sb[:, s4 * 512 : (s4 + 1) * 512],
                                    start=first,
                                    stop=last,
                                )
                if hs[-1] == HPC - 1:
                    for st in range(NST):
                        osb = osb_pool.tile(
                            [P, HQ], f32, tag="osb", name=f"osb_{q4}_{st}"
                        )
                        nc.vector.tensor_copy(out=osb, in_=pos[st])
                        nc.sync.dma_start(
                            out=out[st * P : (st + 1) * P, h_sl], in_=osb
                        )
                    del pos_by_q4[q4]

            emit_part(0, [0, 1])
            emit_part(1, [0, 1])
            emit_recv([2])
            emit_part(0, [2])
            emit_part(1, [2])
            emit_recv([3])
            emit_part(0, [3])
            emit_part(1, [3])
            emit_part(2, [0, 1, 2, 3])
            emit_part(3, [0, 1, 2, 3])
        ph23.close()

    _mark("end")
    global _PHASE_MARKERS
    _PHASE_MARKERS = [
        (n, lo, hi)
        for (n, lo), (_, hi) in zip(_markers, _markers[1:])
    ]
    return nc


def make_in_maps(x, Wq, Wk, Wv, Wo):
    S = x.shape[1]
    xT = np.ascontiguousarray(x.reshape(S, HID).T.astype(np.float32)).astype(BF)
    woT = np.ascontiguousarray(Wo.astype(np.float32).T).astype(BF)

    inv_freq = 1.0 / (
        ROPE_THETA ** (np.arange(0, D, 2, dtype=np.float32) / np.float32(D))
    )
    t = np.arange(S, dtype=np.float32)
    freqs = np.outer(t, inv_freq).astype(np.float32)
    emb = np.concatenate([freqs, freqs], axis=1)
    cosT = np.cos(emb).T.astype(np.float32)  # [D, S]
    sinT = np.sin(emb).T.astype(np.float32)
    # sign-folded sin for the partition-shifted rotate_half:
    #   trot[0:64]  = acc[64:128] * (-sin[0:64])
    #   trot[64:128] = acc[0:64]  *   sin[64:128]
    sinM = np.concatenate([-sinT[0:HD2], sinT[HD2:D]], axis=0)
    scale = np.float32(1.0 / np.sqrt(np.float32(D)))
    cosq = np.ascontiguousarray(cosT * scale).astype(BF)
    sinqm = np.ascontiguousarray(sinM * scale).astype(BF)
    cosk = np.ascontiguousarray(cosT).astype(BF)
    sinkm = np.ascontiguousarray(sinM).astype(BF)

    mask = np.zeros((P, 512), dtype=np.float32)
    for k in range(P):
        mask[k, k:] = 1.0
    maskT = mask.astype(BF)
    ident = np.eye(P, dtype=np.float32).astype(BF)

    in_maps = []
    for m in range(NCORES):
        wqT = np.ascontiguousarray(
            Wq[m * ROWS_Q : (m + 1) * ROWS_Q, :].astype(np.float32).T
        ).astype(BF)
        wkT = np.ascontiguousarray(
            Wk[m * D : (m + 1) * D, :].astype(np.float32).T
        ).astype(BF)
        wvT = np.ascontiguousarray(
            Wv[m * D : (m + 1) * D, :].astype(np.float32).T
        ).astype(BF)
        in_maps.append(
            dict(
                xT=xT,
                wq=wqT,
                wk=wkT,
                wv=wvT,
                wo=woT,
                cosq=cosq,
                sinqm=sinqm,
                cosk=cosk,
                sinkm=sinkm,
                maskT=maskT,
                ident=ident,
            )
        )
    return in_maps


def gather_out(results, S):
    parts = [np.asarray(results[c]["out"], dtype=np.float32) for c in range(NCORES)]
    return np.concatenate(parts, axis=0).reshape(1, S, HID)


def kernel(x, Wq, Wk, Wv, Wo):
    from concourse.bass_utils import run_bass_kernel_spmd

    x = np.asarray(x)
    S = x.shape[1]
    nc = build_nc(S)
    in_maps = make_in_maps(x, np.asarray(Wq), np.asarray(Wk), np.asarray(Wv), np.asarray(Wo))
    res = run_bass_kernel_spmd(nc, in_maps, list(range(NCORES)))
    return gather_out(res.results, S)
